# revision 15
# baseline (speedup 1.0000x reference)
"""Trainium2 Bass kernel for nn_ConvocationV3 (dense_cnn).

Pipeline per sample (B=32, C=384, H=W=54, K=3):
  value = conv1x1(x, w_v) ; qk = pool3x3(conv1x1(x, w_qk)) = conv1x1(pool3x3(x), w_qk)
  h = gelu(conv1x1(qk, w_kg1)) ; kernels = conv1x1(h, w_kg2)
  kernels -= sigmoid(beta)/9 * sum_taps(kernels)
  out = depthwise3x3(value, kernels)  (per-sample, per-channel kernels)
  y = conv1x1(out, w_proj)

Sharding: data-parallel over batch, 4 samples per core on 8 cores.

Engine split (per sample):
  PE   : value conv, qk/kg matmuls, 4 depthwise taps (diag matmul), proj conv
  ACT  : psum evacuations (value->vpad, taps->dw, proj->ych), kdiag, gelu
  DVE  : 4 depthwise taps (STT), vpb shifted copy (4x tensor_copy), small ops
  Pool : pool stage 1 reduce, 1 depthwise tap (STT)
Emission is software-pipelined across samples (stages A/B/C/D) so the PE
stream stays dense enough to hold the high p-state.
"""

import numpy as np
import ml_dtypes

import concourse.bass as bass
import concourse.bacc as bacc
import concourse.mybir as mybir
import concourse.tile as tile
from concourse.bass_utils import run_bass_kernel_spmd

F32 = mybir.dt.float32
BF16 = mybir.dt.bfloat16
AX = mybir.AxisListType
ALU = mybir.AluOpType
ACTF = mybir.ActivationFunctionType

B_LOC = 4          # samples per core
CT = 3             # channel tiles (384 = 3*128)
P = 128
HW = 2916          # 54*54
PW = 56            # padded width/height
BIG = 972          # dma/act chunk (18 rows of 54)
CH = 486           # matmul free chunk (9 rows of 54)
DQ = 96

# taps: t = 3*i + j, flat offset (i-1)*56 + (j-1) on the padded plane.
# PE taps: diag matmuls into psum. DVE taps: tensor_scalar scale (fast
# mode, ~0.33ns/elem) + tensor_tensor add (2x, ~0.57ns/elem) — measured
# faster than the 1x-only scalar_tensor_tensor, and alignment-insensitive.
PE_TAPS = [0, 2, 3, 6, 8]
DVE_TAPS = [1, 4, 5, 7]
DMA_ADD_TAPS = ()  # DMA-accumulate adds: unsupported for bf16 (NaNs on hw)
VPAD_N = 3200      # vpad tile: 2 lead pad elems + 56*56 grid + tail slack


def build_program():
    nc = bacc.Bacc("TRN2", target_bir_lowering=False, debug=False)

    x_d = nc.dram_tensor("x", [B_LOC, CT, P, HW], BF16, kind="ExternalInput").ap()
    wv_d = nc.dram_tensor("wv", [P, CT, 384], BF16, kind="ExternalInput").ap()
    wproj_d = nc.dram_tensor("wproj", [P, CT, 384], BF16, kind="ExternalInput").ap()
    wqk_d = nc.dram_tensor("wqk", [P, CT, 384], F32, kind="ExternalInput").ap()
    wkg1_d = nc.dram_tensor("wkg1", [P, CT, DQ], F32, kind="ExternalInput").ap()
    wkg2e_d = nc.dram_tensor("wkg2e", [DQ + 1, 384], F32, kind="ExternalInput").ap()
    bv_d = nc.dram_tensor("bv", [P, CT], F32, kind="ExternalInput").ap()
    bqk_d = nc.dram_tensor("bqk", [P, CT], F32, kind="ExternalInput").ap()
    bkg1_d = nc.dram_tensor("bkg1", [DQ, 1], F32, kind="ExternalInput").ap()
    bproj_d = nc.dram_tensor("bproj", [P, CT], F32, kind="ExternalInput").ap()
    fac9_d = nc.dram_tensor("fac9", [P, CT], F32, kind="ExternalInput").ap()
    eye_d = nc.dram_tensor("eye", [P, P], BF16, kind="ExternalInput").ap()

    y_d = nc.dram_tensor("y", [B_LOC, CT, P, HW], BF16, kind="ExternalOutput").ap()

    with tile.TileContext(nc) as tc:
        with (
            tc.tile_pool(name="const", bufs=1) as cpool,
            tc.tile_pool(name="xch", bufs=3) as xpool,
            tc.tile_pool(name="tmp", bufs=6) as tpool,
            tc.tile_pool(name="dw", bufs=2) as dwpool,
            tc.tile_pool(name="ych", bufs=4) as ypool,
            tc.tile_pool(name="small", bufs=2) as spool,
            tc.tile_pool(name="mm", bufs=3, space="PSUM") as mmpool,
            tc.tile_pool(name="smallps", bufs=2, space="PSUM") as sppool,
        ):
            # ---- constants ----
            wv = cpool.tile([P, CT, 384], BF16, name="wv_sb")
            wproj = cpool.tile([P, CT, 384], BF16, name="wproj_sb")
            wqk = cpool.tile([P, CT, 384], F32, name="wqk_sb")
            wkg1 = cpool.tile([P, CT, DQ], F32, name="wkg1_sb")
            wkg2e = cpool.tile([DQ + 1, 384], F32, name="wkg2e_sb")
            bv = cpool.tile([P, CT], F32, name="bv_sb")
            bqk = cpool.tile([P, CT], F32, name="bqk_sb")
            bkg1 = cpool.tile([DQ, 1], F32, name="bkg1_sb")
            bproj = cpool.tile([P, CT], F32, name="bproj_sb")
            fac9 = cpool.tile([P, CT], F32, name="fac9_sb")
            eye = cpool.tile([P, P], BF16, name="eye_sb")
            for t_sb, t_dr in [(wv, wv_d), (wproj, wproj_d), (wqk, wqk_d),
                               (wkg1, wkg1_d), (wkg2e, wkg2e_d), (bv, bv_d),
                               (bqk, bqk_d), (bkg1, bkg1_d), (bproj, bproj_d),
                               (fac9, fac9_d), (eye, eye_d)]:
                nc.sync.dma_start(t_sb[:], t_dr[:])

            # HAM warm-up: ~4us of dummy back-to-back matmuls so the PE clock
            # gate opens (K=8/8) before the first real conv burst arrives.
            wps = mmpool.tile([P, 2, 512], F32, name="warm_ps", tag="mm")
            for r in range(12):
                nc.tensor.matmul(wps[:, 0, :384], lhsT=eye[:],
                                 rhs=wv[:, 0, :384],
                                 start=(r == 0), stop=(r == 11))

            # persistent double-buffered padded value planes; borders zeroed
            # ONCE here (interior rewritten per sample, borders never touched)
            vpad_b = [[cpool.tile([P, VPAD_N], BF16, name=f"vpad{i}_{ct}")
                       for ct in range(CT)] for i in range(2)]
            for i in range(2):
                for ct in range(CT):
                    vpv = vpad_b[i][ct][:, 2:2 + PW * PW].rearrange(
                        "p (h w) -> p h w", h=PW)
                    nc.gpsimd.memset(vpv[:, 0:1, :], 0.0)
                    nc.gpsimd.memset(vpv[:, PW - 1:PW, :], 0.0)
                    nc.gpsimd.memset(vpv[:, 1:PW - 1, 0:1], 0.0)
                    nc.gpsimd.memset(vpv[:, 1:PW - 1, PW - 1:PW], 0.0)

            # ---------------- per-sample stage bodies ----------------
            state = {}

            def stage_a(b):
                """x dma, pool1 (Pool), value conv (PE), evac -> vpad (ACT)."""
                vpads = vpad_b[b % 2]
                pool1 = spool.tile([P, CT, 54, 3], F32, name=f"pool1_{b}",
                                   tag="pool1")
                xchs = []
                for g in range(3):
                    xch = xpool.tile([P, CT, BIG], BF16, name=f"x_{b}_{g}",
                                     tag="xch")
                    xchs.append(xch)
                    nc.sync.dma_start(
                        xch[:],
                        x_d[b, :, :, g * BIG:(g + 1) * BIG].transpose([1, 0, 2]))
                for g in range(3):
                    xch = xchs[g]
                    for kt in range(CT):
                        nc.vector.tensor_reduce(
                            out=pool1[:, kt, g * 18:(g + 1) * 18, :],
                            in_=xch[:, kt].rearrange(
                                "p (h wb w) -> p h wb w", wb=3, w=18),
                            axis=AX.X, op=ALU.add)
                    for mt in range(CT):
                        ps = mmpool.tile([P, 2, 512], F32,
                                         name=f"vps_{b}_{g}_{mt}", tag="mm")
                        for kt in range(CT):
                            for s in range(2):
                                nc.tensor.matmul(
                                    ps[:, s, :CH],
                                    lhsT=wv[:, kt, mt * P:(mt + 1) * P],
                                    rhs=xch[:, kt, s * CH:(s + 1) * CH],
                                    start=(kt == 0), stop=(kt == CT - 1))
                        nc.scalar.activation(
                            out=vpads[mt][:, 2:2 + PW * PW].rearrange(
                                "p (h w) -> p h w", h=PW)[
                                :, 1 + g * 18:1 + (g + 1) * 18, 1:55],
                            in_=ps[:, :, :CH],
                            func=ACTF.Identity, bias=bv[:, mt:mt + 1], scale=1.0)
                state[b] = {"pool1": pool1}

            def stage_b(b):
                """kernel-gen chain + kdiag (ACT)."""
                pool1 = state[b]["pool1"]
                pooled = spool.tile([P, CT, 9], F32, name=f"pooled_{b}",
                                    tag="pooled")
                for kt in range(CT):
                    nc.vector.tensor_reduce(
                        out=pooled[:, kt].rearrange("p (hb wb) -> p hb wb", hb=3),
                        in_=pool1[:, kt].rearrange(
                            "p (hb hs) wb -> p hb wb hs", hb=3),
                        axis=AX.X, op=ALU.add)

                qk = spool.tile([P, CT, 9], F32, name=f"qk_{b}", tag="qk")
                for mt in range(CT):
                    psq = sppool.tile([P, 9], F32, name=f"qps_{b}_{mt}",
                                      tag="sps")
                    for kt in range(CT):
                        nc.tensor.matmul(
                            psq[:],
                            lhsT=wqk[:, kt, mt * P:(mt + 1) * P],
                            rhs=pooled[:, kt],
                            start=(kt == 0), stop=(kt == CT - 1))
                    nc.scalar.activation(out=qk[:, mt], in_=psq[:],
                                         func=ACTF.Identity,
                                         bias=bqk[:, mt:mt + 1], scale=1.0)

                hsb = spool.tile([DQ + 1, 9], F32, name=f"h_{b}", tag="h")
                psh = sppool.tile([DQ, 9], F32, name=f"hps_{b}", tag="sps")
                for kt in range(CT):
                    nc.tensor.matmul(
                        psh[:],
                        lhsT=wkg1[:, kt, :],
                        rhs=qk[:, kt],
                        start=(kt == 0), stop=(kt == CT - 1))
                nc.scalar.activation(out=hsb[:DQ, :], in_=psh[:], func=ACTF.Gelu,
                                     bias=bkg1[:, 0:1], scale=1.0)
                nc.gpsimd.memset(hsb[DQ:DQ + 1, :], 1.0)  # bias row for kg2

                ksb = spool.tile([P, CT, 9], F32, name=f"k_{b}", tag="ksb")
                ksum = spool.tile([P, CT], F32, name=f"ksum_{b}", tag="ksum")
                for mt in range(CT):
                    psk = sppool.tile([P, 9], F32, name=f"kps_{b}_{mt}",
                                      tag="sps")
                    nc.tensor.matmul(
                        psk[:],
                        lhsT=wkg2e[:, mt * P:(mt + 1) * P],
                        rhs=hsb[:],
                        start=True, stop=True)
                    nc.vector.tensor_reduce(out=ksum[:, mt:mt + 1], in_=psk[:],
                                            axis=AX.X, op=ALU.add)
                    nc.vector.tensor_scalar(
                        out=ksum[:, mt:mt + 1], in0=ksum[:, mt:mt + 1],
                        scalar1=fac9[:, mt:mt + 1], scalar2=None, op0=ALU.mult)
                    nc.vector.tensor_scalar(
                        out=ksb[:, mt], in0=psk[:],
                        scalar1=ksum[:, mt:mt + 1], scalar2=None,
                        op0=ALU.subtract)

                # diag(k) for PE taps — on ACT (Copy with per-partition scale)
                kdiag = spool.tile([P, CT, len(PE_TAPS), P], BF16,
                                   name=f"kd_{b}", tag="kdiag")
                for ct in range(CT):
                    for ti, t in enumerate(PE_TAPS):
                        nc.scalar.activation(
                            out=kdiag[:, ct, ti], in_=eye[:],
                            func=ACTF.Copy, bias=0.0,
                            scale=ksb[:, ct, t:t + 1])

                state[b].update({"ksb": ksb, "kdiag": kdiag})

            def stage_c(b):
                """depthwise: PE diag taps -> psum -> ACT -> dw; DVE taps as
                tensor_scalar scale into tmp + tensor_tensor add into dw."""
                vpads = vpad_b[b % 2]
                ksb = state[b]["ksb"]
                kdiag = state[b]["kdiag"]
                dw = dwpool.tile([P, CT, PW * PW], BF16, name=f"dw_{b}",
                                 tag="dw")
                tmps = {}
                for ct in range(CT):
                    vpv = vpads[ct][:, 2:2 + PW * PW].rearrange(
                        "p (h w) -> p h w", h=PW)
                    for g in range(3):
                        dps = mmpool.tile([P, 2, 512], F32,
                                          name=f"dps_{b}_{ct}_{g}", tag="mm")
                        for ti, t in enumerate(PE_TAPS):
                            i, j = divmod(t, 3)
                            for s2 in range(2):
                                ch = g * 2 + s2
                                nc.tensor.matmul(
                                    dps[:, s2, :CH],
                                    lhsT=kdiag[:, ct, ti],
                                    rhs=vpv[:, ch * 9 + i: ch * 9 + i + 9,
                                            j:j + 54],
                                    start=(ti == 0),
                                    stop=(ti == len(PE_TAPS) - 1))
                        nc.scalar.activation(
                            out=dw[:, ct].rearrange("p (h w) -> p h w", h=PW)[
                                :, 1 + g * 18:1 + (g + 1) * 18, 1:55],
                            in_=dps[:, :, :CH],
                            func=ACTF.Copy, bias=0.0, scale=1.0)
                    # scales first (only need vpad; overlap with ACT evacs)
                    for t in DVE_TAPS:
                        i, j = divmod(t, 3)
                        off = (i - 1) * PW + (j - 1)
                        a0 = max(0, -off)
                        ln = PW * PW - a0
                        tmp = tpool.tile([P, PW * PW], BF16,
                                         name=f"tmp_{b}_{ct}_{t}", tag="tmp")
                        nc.vector.tensor_scalar(
                            out=tmp[:, :ln],
                            in0=vpads[ct][:, 2 + a0 + off: 2 + a0 + off + ln],
                            scalar1=ksb[:, ct, t:t + 1], scalar2=None,
                            op0=ALU.mult)
                        tmps[(ct, t)] = (tmp, a0, ln)
                # adds after evacs: dw[ct] += tmp. Taps in DMA_ADD_TAPS go
                # through the gpsimd software-DGE DMA accumulate path (frees
                # DVE cycles; DMA engines are otherwise ~35% busy).
                for ct in range(CT):
                    for t in DVE_TAPS:
                        tmp, a0, ln = tmps[(ct, t)]
                        if t in DMA_ADD_TAPS:
                            nc.gpsimd.dma_start(
                                dw[:, ct, a0:a0 + ln], tmp[:, :ln],
                                accum_op=ALU.add)
                        else:
                            nc.vector.tensor_tensor(
                                out=dw[:, ct, a0:a0 + ln],
                                in0=dw[:, ct, a0:a0 + ln],
                                in1=tmp[:, :ln], op=ALU.add)
                state[b]["dw"] = dw

            def stage_d(b):
                """proj conv (PE) -> ACT (bias, bf16) -> dma out."""
                dw = state[b]["dw"]
                for g in range(3):
                    for mt in range(CT):
                        ps = mmpool.tile([P, 2, 512], F32,
                                         name=f"pps_{b}_{mt}_{g}", tag="mm")
                        for kt in range(CT):
                            for s in range(2):
                                ch = g * 2 + s
                                nc.tensor.matmul(
                                    ps[:, s, :CH],
                                    lhsT=wproj[:, kt, mt * P:(mt + 1) * P],
                                    rhs=dw[:, kt].rearrange(
                                        "p (h w) -> p h w", h=PW)[
                                        :, ch * 9 + 1: ch * 9 + 10, 1:55],
                                    start=(kt == 0), stop=(kt == CT - 1))
                        ych = ypool.tile([P, BIG], BF16, name=f"y_{b}_{mt}_{g}",
                                         tag="ych")
                        nc.scalar.activation(out=ych[:], in_=ps[:, :, :CH],
                                             func=ACTF.Identity,
                                             bias=bproj[:, mt:mt + 1], scale=1.0)
                        nc.sync.dma_start(
                            y_d[b, mt, :, g * BIG:(g + 1) * BIG], ych[:])

            # ---------------- software-pipelined emission ----------------
            # B(b) sits right before C(b) so its small serial qk chain never
            # head-of-line blocks a proj stage in the in-order PE queue.
            stage_a(0)
            stage_b(0)
            stage_a(1)
            stage_c(0)
            stage_a(2)
            stage_d(0)
            stage_b(1)
            stage_c(1)
            stage_a(3)
            stage_d(1)
            stage_b(2)
            stage_c(2)
            stage_d(2)
            stage_b(3)
            stage_c(3)
            stage_d(3)
    nc.compile()
    return nc


def _prep_inputs(x, w_qk, b_qk, w_kg1, b_kg1, w_kg2, b_kg2, w_v, b_v,
                 w_proj, b_proj, beta):
    bf = ml_dtypes.bfloat16
    f32 = np.float32

    def lay_w(w, dt):  # (O, Cin) -> lhsT layout [p, kt, O]
        wt = np.ascontiguousarray(w.T.reshape(CT, P, -1).transpose(1, 0, 2))
        return wt.astype(dt)

    def lay_b(v):  # (C,) -> [p, ct]
        return np.ascontiguousarray(v.reshape(CT, P).T).astype(f32)

    consts = {
        "wv": lay_w(w_v, bf),
        "wproj": lay_w(w_proj, bf),
        "wqk": lay_w(w_qk / 324.0, f32),
        "wkg1": lay_w(w_kg1, f32),
        "wkg2e": np.ascontiguousarray(
            np.vstack([w_kg2.T, b_kg2[None, :]])).astype(f32),
        "bv": lay_b(b_v),
        "bqk": lay_b(b_qk),
        "bkg1": np.ascontiguousarray(b_kg1.reshape(DQ, 1)).astype(f32),
        "bproj": lay_b(b_proj),
        "fac9": lay_b(1.0 / (1.0 + np.exp(-beta.astype(np.float64))) / 9.0),
        "eye": np.eye(P, dtype=bf),
    }
    xs = np.ascontiguousarray(
        x.reshape(8, B_LOC, CT, P, HW)).astype(bf)
    in_maps = [dict(consts, x=np.ascontiguousarray(xs[c])) for c in range(8)]
    return in_maps


_CACHED_NC = None


def kernel(**inputs):
    global _CACHED_NC
    in_maps = _prep_inputs(**{k: np.asarray(v) for k, v in inputs.items()})
    if _CACHED_NC is None:
        _CACHED_NC = build_program()
    res = run_bass_kernel_spmd(_CACHED_NC, in_maps, core_ids=list(range(8)))
    ys = np.stack([r["y"] for r in res.results])  # (8, 4, 3, 128, 2916)
    return ys.reshape(32, 384, 54, 54).astype(np.float32)


# revision 16
# speedup vs baseline: 1.0171x; 1.0171x over previous
"""Trainium2 Bass kernel for nn_ConvocationV3 (dense_cnn).

Pipeline per sample (B=32, C=384, H=W=54, K=3):
  value = conv1x1(x, w_v) ; qk = pool3x3(conv1x1(x, w_qk)) = conv1x1(pool3x3(x), w_qk)
  h = gelu(conv1x1(qk, w_kg1)) ; kernels = conv1x1(h, w_kg2)
  kernels -= sigmoid(beta)/9 * sum_taps(kernels)
  out = depthwise3x3(value, kernels)  (per-sample, per-channel kernels)
  y = conv1x1(out, w_proj)

Sharding: data-parallel over batch, 4 samples per core on 8 cores.

Engine split (per sample):
  PE   : value conv, qk/kg matmuls, 4 depthwise taps (diag matmul), proj conv
  ACT  : psum evacuations (value->vpad, taps->dw, proj->ych), kdiag, gelu
  DVE  : 4 depthwise taps (STT), vpb shifted copy (4x tensor_copy), small ops
  Pool : pool stage 1 reduce, 1 depthwise tap (STT)
Emission is software-pipelined across samples (stages A/B/C/D) so the PE
stream stays dense enough to hold the high p-state.
"""

import numpy as np
import ml_dtypes

import concourse.bass as bass
import concourse.bacc as bacc
import concourse.mybir as mybir
import concourse.tile as tile
from concourse.bass_utils import run_bass_kernel_spmd

F32 = mybir.dt.float32
BF16 = mybir.dt.bfloat16
AX = mybir.AxisListType
ALU = mybir.AluOpType
ACTF = mybir.ActivationFunctionType

B_LOC = 4          # samples per core
CT = 3             # channel tiles (384 = 3*128)
P = 128
HW = 2916          # 54*54
PW = 56            # padded width/height
BIG = 972          # dma/act chunk (18 rows of 54)
CH = 486           # matmul free chunk (9 rows of 54)
DQ = 96

# taps: t = 3*i + j, flat offset (i-1)*56 + (j-1) on the padded plane.
# PE taps: diag matmuls into psum. DVE taps: tensor_scalar scale (fast
# mode, ~0.33ns/elem) + tensor_tensor add (2x, ~0.57ns/elem) — measured
# faster than the 1x-only scalar_tensor_tensor, and alignment-insensitive.
PE_TAPS = [0, 2, 3, 6, 8]
DVE_TAPS = [1, 4, 5, 7]
DMA_ADD_TAPS = ()  # DMA-accumulate adds: unsupported for bf16 (NaNs on hw)
VPAD_N = 3200      # vpad tile: 2 lead pad elems + 56*56 grid + tail slack


def build_program():
    nc = bacc.Bacc("TRN2", target_bir_lowering=False, debug=False)

    x_d = nc.dram_tensor("x", [B_LOC, CT, P, HW], BF16, kind="ExternalInput").ap()
    wv_d = nc.dram_tensor("wv", [P, CT, 384], BF16, kind="ExternalInput").ap()
    wproj_d = nc.dram_tensor("wproj", [P, CT, 384], BF16, kind="ExternalInput").ap()
    wqk_d = nc.dram_tensor("wqk", [P, CT, 384], F32, kind="ExternalInput").ap()
    wkg1_d = nc.dram_tensor("wkg1", [P, CT, DQ], F32, kind="ExternalInput").ap()
    wkg2e_d = nc.dram_tensor("wkg2e", [DQ + 1, 384], F32, kind="ExternalInput").ap()
    bv_d = nc.dram_tensor("bv", [P, CT], F32, kind="ExternalInput").ap()
    bqk_d = nc.dram_tensor("bqk", [P, CT], F32, kind="ExternalInput").ap()
    bkg1_d = nc.dram_tensor("bkg1", [DQ, 1], F32, kind="ExternalInput").ap()
    bproj_d = nc.dram_tensor("bproj", [P, CT], F32, kind="ExternalInput").ap()
    fac9_d = nc.dram_tensor("fac9", [P, CT], F32, kind="ExternalInput").ap()
    eye_d = nc.dram_tensor("eye", [P, P], BF16, kind="ExternalInput").ap()

    y_d = nc.dram_tensor("y", [B_LOC, CT, P, HW], BF16, kind="ExternalOutput").ap()

    with tile.TileContext(nc) as tc:
        with (
            tc.tile_pool(name="const", bufs=1) as cpool,
            tc.tile_pool(name="xch", bufs=3) as xpool,
            tc.tile_pool(name="tmp", bufs=6) as tpool,
            tc.tile_pool(name="dw", bufs=2) as dwpool,
            tc.tile_pool(name="ych", bufs=4) as ypool,
            tc.tile_pool(name="small", bufs=2) as spool,
            tc.tile_pool(name="mm", bufs=3, space="PSUM") as mmpool,
            tc.tile_pool(name="smallps", bufs=2, space="PSUM") as sppool,
        ):
            # ---- constants ----
            wv = cpool.tile([P, CT, 384], BF16, name="wv_sb")
            wproj = cpool.tile([P, CT, 384], BF16, name="wproj_sb")
            wqk = cpool.tile([P, CT, 384], F32, name="wqk_sb")
            wkg1 = cpool.tile([P, CT, DQ], F32, name="wkg1_sb")
            wkg2e = cpool.tile([DQ + 1, 384], F32, name="wkg2e_sb")
            bv = cpool.tile([P, CT], F32, name="bv_sb")
            bqk = cpool.tile([P, CT], F32, name="bqk_sb")
            bkg1 = cpool.tile([DQ, 1], F32, name="bkg1_sb")
            bproj = cpool.tile([P, CT], F32, name="bproj_sb")
            fac9 = cpool.tile([P, CT], F32, name="fac9_sb")
            eye = cpool.tile([P, P], BF16, name="eye_sb")
            for t_sb, t_dr in [(wv, wv_d), (wproj, wproj_d), (wqk, wqk_d),
                               (wkg1, wkg1_d), (wkg2e, wkg2e_d), (bv, bv_d),
                               (bqk, bqk_d), (bkg1, bkg1_d), (bproj, bproj_d),
                               (fac9, fac9_d), (eye, eye_d)]:
                nc.sync.dma_start(t_sb[:], t_dr[:])

            # persistent double-buffered padded value planes; borders zeroed
            # ONCE here (interior rewritten per sample, borders never touched)
            vpad_b = [[cpool.tile([P, VPAD_N], BF16, name=f"vpad{i}_{ct}")
                       for ct in range(CT)] for i in range(2)]
            for i in range(2):
                for ct in range(CT):
                    vpv = vpad_b[i][ct][:, 2:2 + PW * PW].rearrange(
                        "p (h w) -> p h w", h=PW)
                    nc.gpsimd.memset(vpv[:, 0:1, :], 0.0)
                    nc.gpsimd.memset(vpv[:, PW - 1:PW, :], 0.0)
                    nc.gpsimd.memset(vpv[:, 1:PW - 1, 0:1], 0.0)
                    nc.gpsimd.memset(vpv[:, 1:PW - 1, PW - 1:PW], 0.0)

            # ---------------- per-sample stage bodies ----------------
            state = {}

            def stage_a(b):
                """x dma, pool1 (Pool), value conv (PE), evac -> vpad (ACT)."""
                vpads = vpad_b[b % 2]
                pool1 = spool.tile([P, CT, 54, 3], F32, name=f"pool1_{b}",
                                   tag="pool1")
                xchs = []
                for g in range(3):
                    xch = xpool.tile([P, CT, BIG], BF16, name=f"x_{b}_{g}",
                                     tag="xch")
                    xchs.append(xch)
                    nc.sync.dma_start(
                        xch[:],
                        x_d[b, :, :, g * BIG:(g + 1) * BIG].transpose([1, 0, 2]))
                for g in range(3):
                    xch = xchs[g]
                    for kt in range(CT):
                        nc.vector.tensor_reduce(
                            out=pool1[:, kt, g * 18:(g + 1) * 18, :],
                            in_=xch[:, kt].rearrange(
                                "p (h wb w) -> p h wb w", wb=3, w=18),
                            axis=AX.X, op=ALU.add)
                    for mt in range(CT):
                        ps = mmpool.tile([P, 2, 512], F32,
                                         name=f"vps_{b}_{g}_{mt}", tag="mm")
                        for kt in range(CT):
                            for s in range(2):
                                nc.tensor.matmul(
                                    ps[:, s, :CH],
                                    lhsT=wv[:, kt, mt * P:(mt + 1) * P],
                                    rhs=xch[:, kt, s * CH:(s + 1) * CH],
                                    start=(kt == 0), stop=(kt == CT - 1))
                        nc.scalar.activation(
                            out=vpads[mt][:, 2:2 + PW * PW].rearrange(
                                "p (h w) -> p h w", h=PW)[
                                :, 1 + g * 18:1 + (g + 1) * 18, 1:55],
                            in_=ps[:, :, :CH],
                            func=ACTF.Identity, bias=bv[:, mt:mt + 1], scale=1.0)
                state[b] = {"pool1": pool1}

            def stage_b(b):
                """kernel-gen chain + kdiag (ACT)."""
                pool1 = state[b]["pool1"]
                pooled = spool.tile([P, CT, 9], F32, name=f"pooled_{b}",
                                    tag="pooled")
                for kt in range(CT):
                    nc.vector.tensor_reduce(
                        out=pooled[:, kt].rearrange("p (hb wb) -> p hb wb", hb=3),
                        in_=pool1[:, kt].rearrange(
                            "p (hb hs) wb -> p hb wb hs", hb=3),
                        axis=AX.X, op=ALU.add)

                qk = spool.tile([P, CT, 9], F32, name=f"qk_{b}", tag="qk")
                for mt in range(CT):
                    psq = sppool.tile([P, 9], F32, name=f"qps_{b}_{mt}",
                                      tag="sps")
                    for kt in range(CT):
                        nc.tensor.matmul(
                            psq[:],
                            lhsT=wqk[:, kt, mt * P:(mt + 1) * P],
                            rhs=pooled[:, kt],
                            start=(kt == 0), stop=(kt == CT - 1))
                    nc.scalar.activation(out=qk[:, mt], in_=psq[:],
                                         func=ACTF.Identity,
                                         bias=bqk[:, mt:mt + 1], scale=1.0)

                hsb = spool.tile([DQ + 1, 9], F32, name=f"h_{b}", tag="h")
                psh = sppool.tile([DQ, 9], F32, name=f"hps_{b}", tag="sps")
                for kt in range(CT):
                    nc.tensor.matmul(
                        psh[:],
                        lhsT=wkg1[:, kt, :],
                        rhs=qk[:, kt],
                        start=(kt == 0), stop=(kt == CT - 1))
                nc.scalar.activation(out=hsb[:DQ, :], in_=psh[:], func=ACTF.Gelu,
                                     bias=bkg1[:, 0:1], scale=1.0)
                nc.gpsimd.memset(hsb[DQ:DQ + 1, :], 1.0)  # bias row for kg2

                ksb = spool.tile([P, CT, 9], F32, name=f"k_{b}", tag="ksb")
                ksum = spool.tile([P, CT], F32, name=f"ksum_{b}", tag="ksum")
                for mt in range(CT):
                    psk = sppool.tile([P, 9], F32, name=f"kps_{b}_{mt}",
                                      tag="sps")
                    nc.tensor.matmul(
                        psk[:],
                        lhsT=wkg2e[:, mt * P:(mt + 1) * P],
                        rhs=hsb[:],
                        start=True, stop=True)
                    nc.vector.tensor_reduce(out=ksum[:, mt:mt + 1], in_=psk[:],
                                            axis=AX.X, op=ALU.add)
                    nc.vector.tensor_scalar(
                        out=ksum[:, mt:mt + 1], in0=ksum[:, mt:mt + 1],
                        scalar1=fac9[:, mt:mt + 1], scalar2=None, op0=ALU.mult)
                    nc.vector.tensor_scalar(
                        out=ksb[:, mt], in0=psk[:],
                        scalar1=ksum[:, mt:mt + 1], scalar2=None,
                        op0=ALU.subtract)

                # diag(k) for PE taps — on ACT (Copy with per-partition scale)
                kdiag = spool.tile([P, CT, len(PE_TAPS), P], BF16,
                                   name=f"kd_{b}", tag="kdiag")
                for ct in range(CT):
                    for ti, t in enumerate(PE_TAPS):
                        nc.scalar.activation(
                            out=kdiag[:, ct, ti], in_=eye[:],
                            func=ACTF.Copy, bias=0.0,
                            scale=ksb[:, ct, t:t + 1])

                state[b].update({"ksb": ksb, "kdiag": kdiag})

            def stage_c(b):
                """depthwise: PE diag taps -> psum -> ACT -> dw; DVE taps as
                tensor_scalar scale into tmp + tensor_tensor add into dw."""
                vpads = vpad_b[b % 2]
                ksb = state[b]["ksb"]
                kdiag = state[b]["kdiag"]
                dw = dwpool.tile([P, CT, PW * PW], BF16, name=f"dw_{b}",
                                 tag="dw")
                tmps = {}
                for ct in range(CT):
                    vpv = vpads[ct][:, 2:2 + PW * PW].rearrange(
                        "p (h w) -> p h w", h=PW)
                    for g in range(3):
                        dps = mmpool.tile([P, 2, 512], F32,
                                          name=f"dps_{b}_{ct}_{g}", tag="mm")
                        for ti, t in enumerate(PE_TAPS):
                            i, j = divmod(t, 3)
                            for s2 in range(2):
                                ch = g * 2 + s2
                                nc.tensor.matmul(
                                    dps[:, s2, :CH],
                                    lhsT=kdiag[:, ct, ti],
                                    rhs=vpv[:, ch * 9 + i: ch * 9 + i + 9,
                                            j:j + 54],
                                    start=(ti == 0),
                                    stop=(ti == len(PE_TAPS) - 1))
                        nc.scalar.activation(
                            out=dw[:, ct].rearrange("p (h w) -> p h w", h=PW)[
                                :, 1 + g * 18:1 + (g + 1) * 18, 1:55],
                            in_=dps[:, :, :CH],
                            func=ACTF.Copy, bias=0.0, scale=1.0)
                    # scales first (only need vpad; overlap with ACT evacs)
                    for t in DVE_TAPS:
                        i, j = divmod(t, 3)
                        off = (i - 1) * PW + (j - 1)
                        a0 = max(0, -off)
                        ln = PW * PW - a0
                        tmp = tpool.tile([P, PW * PW], BF16,
                                         name=f"tmp_{b}_{ct}_{t}", tag="tmp")
                        nc.vector.tensor_scalar(
                            out=tmp[:, :ln],
                            in0=vpads[ct][:, 2 + a0 + off: 2 + a0 + off + ln],
                            scalar1=ksb[:, ct, t:t + 1], scalar2=None,
                            op0=ALU.mult)
                        tmps[(ct, t)] = (tmp, a0, ln)
                # adds after evacs: dw[ct] += tmp. Taps in DMA_ADD_TAPS go
                # through the gpsimd software-DGE DMA accumulate path (frees
                # DVE cycles; DMA engines are otherwise ~35% busy).
                for ct in range(CT):
                    for t in DVE_TAPS:
                        tmp, a0, ln = tmps[(ct, t)]
                        if t in DMA_ADD_TAPS:
                            nc.gpsimd.dma_start(
                                dw[:, ct, a0:a0 + ln], tmp[:, :ln],
                                accum_op=ALU.add)
                        else:
                            nc.vector.tensor_tensor(
                                out=dw[:, ct, a0:a0 + ln],
                                in0=dw[:, ct, a0:a0 + ln],
                                in1=tmp[:, :ln], op=ALU.add)
                state[b]["dw"] = dw

            def stage_d(b):
                """proj conv (PE) -> ACT (bias, bf16) -> dma out."""
                dw = state[b]["dw"]
                for g in range(3):
                    for mt in range(CT):
                        ps = mmpool.tile([P, 2, 512], F32,
                                         name=f"pps_{b}_{mt}_{g}", tag="mm")
                        for kt in range(CT):
                            for s in range(2):
                                ch = g * 2 + s
                                nc.tensor.matmul(
                                    ps[:, s, :CH],
                                    lhsT=wproj[:, kt, mt * P:(mt + 1) * P],
                                    rhs=dw[:, kt].rearrange(
                                        "p (h w) -> p h w", h=PW)[
                                        :, ch * 9 + 1: ch * 9 + 10, 1:55],
                                    start=(kt == 0), stop=(kt == CT - 1))
                        ych = ypool.tile([P, BIG], BF16, name=f"y_{b}_{mt}_{g}",
                                         tag="ych")
                        nc.scalar.activation(out=ych[:], in_=ps[:, :, :CH],
                                             func=ACTF.Identity,
                                             bias=bproj[:, mt:mt + 1], scale=1.0)
                        nc.sync.dma_start(
                            y_d[b, mt, :, g * BIG:(g + 1) * BIG], ych[:])

            # ---------------- software-pipelined emission ----------------
            # B(b) sits right before C(b) so its small serial qk chain never
            # head-of-line blocks a proj stage in the in-order PE queue.
            stage_a(0)
            stage_b(0)
            stage_a(1)
            stage_c(0)
            stage_a(2)
            stage_d(0)
            stage_b(1)
            stage_c(1)
            stage_a(3)
            stage_d(1)
            stage_b(2)
            stage_c(2)
            stage_d(2)
            stage_b(3)
            stage_c(3)
            stage_d(3)
    nc.compile()
    return nc


def _prep_inputs(x, w_qk, b_qk, w_kg1, b_kg1, w_kg2, b_kg2, w_v, b_v,
                 w_proj, b_proj, beta):
    bf = ml_dtypes.bfloat16
    f32 = np.float32

    def lay_w(w, dt):  # (O, Cin) -> lhsT layout [p, kt, O]
        wt = np.ascontiguousarray(w.T.reshape(CT, P, -1).transpose(1, 0, 2))
        return wt.astype(dt)

    def lay_b(v):  # (C,) -> [p, ct]
        return np.ascontiguousarray(v.reshape(CT, P).T).astype(f32)

    consts = {
        "wv": lay_w(w_v, bf),
        "wproj": lay_w(w_proj, bf),
        "wqk": lay_w(w_qk / 324.0, f32),
        "wkg1": lay_w(w_kg1, f32),
        "wkg2e": np.ascontiguousarray(
            np.vstack([w_kg2.T, b_kg2[None, :]])).astype(f32),
        "bv": lay_b(b_v),
        "bqk": lay_b(b_qk),
        "bkg1": np.ascontiguousarray(b_kg1.reshape(DQ, 1)).astype(f32),
        "bproj": lay_b(b_proj),
        "fac9": lay_b(1.0 / (1.0 + np.exp(-beta.astype(np.float64))) / 9.0),
        "eye": np.eye(P, dtype=bf),
    }
    xs = np.ascontiguousarray(
        x.reshape(8, B_LOC, CT, P, HW)).astype(bf)
    in_maps = [dict(consts, x=np.ascontiguousarray(xs[c])) for c in range(8)]
    return in_maps


_CACHED_NC = None


def kernel(**inputs):
    global _CACHED_NC
    in_maps = _prep_inputs(**{k: np.asarray(v) for k, v in inputs.items()})
    if _CACHED_NC is None:
        _CACHED_NC = build_program()
    res = run_bass_kernel_spmd(_CACHED_NC, in_maps, core_ids=list(range(8)))
    ys = np.stack([r["y"] for r in res.results])  # (8, 4, 3, 128, 2916)
    return ys.reshape(32, 384, 54, 54).astype(np.float32)


# revision 22
# speedup vs baseline: 1.0587x; 1.0409x over previous
"""Trainium2 Bass kernel for nn_ConvocationV3 (dense_cnn).

Pipeline per sample (B=32, C=384, H=W=54, K=3):
  value = conv1x1(x, w_v) ; qk = pool3x3(conv1x1(x, w_qk)) = conv1x1(pool3x3(x), w_qk)
  h = gelu(conv1x1(qk, w_kg1)) ; kernels = conv1x1(h, w_kg2)
  kernels -= sigmoid(beta)/9 * sum_taps(kernels)
  out = depthwise3x3(value, kernels)  (per-sample, per-channel kernels)
  y = conv1x1(out, w_proj)

Sharding: data-parallel over batch, 4 samples per core on 8 cores.

Engine split (per sample):
  PE   : value conv, qk/kg matmuls, 4 depthwise taps (diag matmul), proj conv
  ACT  : psum evacuations (value->vpad, taps->dw, proj->ych), kdiag, gelu
  DVE  : 4 depthwise taps (STT), vpb shifted copy (4x tensor_copy), small ops
  Pool : pool stage 1 reduce, 1 depthwise tap (STT)
Emission is software-pipelined across samples (stages A/B/C/D) so the PE
stream stays dense enough to hold the high p-state.
"""

import numpy as np
import ml_dtypes

import concourse.bass as bass
import concourse.bacc as bacc
import concourse.mybir as mybir
import concourse.tile as tile
from concourse.bass_utils import run_bass_kernel_spmd

F32 = mybir.dt.float32
BF16 = mybir.dt.bfloat16
AX = mybir.AxisListType
ALU = mybir.AluOpType
ACTF = mybir.ActivationFunctionType

B_LOC = 4          # samples per core
CT = 3             # channel tiles (384 = 3*128)
P = 128
HW = 2916          # 54*54
PW = 56            # padded width/height
BIG = 972          # dma/act chunk (18 rows of 54)
CH = 486           # matmul free chunk (9 rows of 54)
DQ = 96

# taps: t = 3*i + j, flat offset (i-1)*56 + (j-1) on the padded plane.
# PE taps: diag matmuls into psum. DVE taps: tensor_scalar scale (fast
# mode, ~0.33ns/elem) + tensor_tensor add (2x, ~0.57ns/elem) — measured
# faster than the 1x-only scalar_tensor_tensor, and alignment-insensitive.
PE_TAPS = [0, 2, 3, 6, 8]
DVE_TAPS = [1, 4, 5, 7]
DMA_ADD_TAPS = ()  # DMA-accumulate adds: unsupported for bf16 (NaNs on hw)
VPAD_N = 3200      # vpad tile: 2 lead pad elems + 56*56 grid + tail slack


def build_program():
    nc = bacc.Bacc("TRN2", target_bir_lowering=False, debug=False)

    x_d = nc.dram_tensor("x", [B_LOC, CT, P, HW], BF16, kind="ExternalInput").ap()
    wv_d = nc.dram_tensor("wv", [P, CT, 384], BF16, kind="ExternalInput").ap()
    wproj_d = nc.dram_tensor("wproj", [P, CT, 384], BF16, kind="ExternalInput").ap()
    wqk_d = nc.dram_tensor("wqk", [P, CT, 384], F32, kind="ExternalInput").ap()
    wkg1_d = nc.dram_tensor("wkg1", [P, CT, DQ], F32, kind="ExternalInput").ap()
    wkg2e_d = nc.dram_tensor("wkg2e", [DQ + 1, 384], F32, kind="ExternalInput").ap()
    bv_d = nc.dram_tensor("bv", [P, CT], F32, kind="ExternalInput").ap()
    bqk_d = nc.dram_tensor("bqk", [P, CT], F32, kind="ExternalInput").ap()
    bkg1_d = nc.dram_tensor("bkg1", [DQ, 1], F32, kind="ExternalInput").ap()
    bproj_d = nc.dram_tensor("bproj", [P, CT], F32, kind="ExternalInput").ap()
    fac9_d = nc.dram_tensor("fac9", [P, CT], F32, kind="ExternalInput").ap()
    eye_d = nc.dram_tensor("eye", [P, P], BF16, kind="ExternalInput").ap()

    y_d = nc.dram_tensor("y", [B_LOC, CT, P, HW], BF16, kind="ExternalOutput").ap()

    with tile.TileContext(nc) as tc:
        with (
            tc.tile_pool(name="const", bufs=1) as cpool,
            tc.tile_pool(name="xch", bufs=3) as xpool,
            tc.tile_pool(name="tmp", bufs=6) as tpool,
            tc.tile_pool(name="dw", bufs=2) as dwpool,
            tc.tile_pool(name="ych", bufs=4) as ypool,
            tc.tile_pool(name="small", bufs=2) as spool,
            tc.tile_pool(name="mm", bufs=3, space="PSUM") as mmpool,
            tc.tile_pool(name="smallps", bufs=2, space="PSUM") as sppool,
        ):
            # ---- constants ----
            wv = cpool.tile([P, CT, 384], BF16, name="wv_sb")
            wproj = cpool.tile([P, CT, 384], BF16, name="wproj_sb")
            wqk = cpool.tile([P, CT, 384], F32, name="wqk_sb")
            wkg1 = cpool.tile([P, CT, DQ], F32, name="wkg1_sb")
            wkg2e = cpool.tile([DQ + 1, 384], F32, name="wkg2e_sb")
            bv = cpool.tile([P, CT], F32, name="bv_sb")
            bqk = cpool.tile([P, CT], F32, name="bqk_sb")
            bkg1 = cpool.tile([DQ, 1], F32, name="bkg1_sb")
            bproj = cpool.tile([P, CT], F32, name="bproj_sb")
            fac9 = cpool.tile([P, CT], F32, name="fac9_sb")
            eye = cpool.tile([P, P], BF16, name="eye_sb")
            # only wv/bv gate the first value-conv burst; the rest of the
            # consts are DMA'd after sample 0's x chunks (SP queue is FIFO,
            # so front-loading them would delay time-to-first-matmul)
            late_consts = [(wproj, wproj_d), (wqk, wqk_d), (wkg1, wkg1_d),
                           (wkg2e, wkg2e_d), (bqk, bqk_d), (bkg1, bkg1_d),
                           (bproj, bproj_d), (fac9, fac9_d), (eye, eye_d)]
            for t_sb, t_dr in [(wv, wv_d), (bv, bv_d)]:
                nc.sync.dma_start(t_sb[:], t_dr[:])

            # persistent double-buffered padded value planes; borders zeroed
            # ONCE here (interior rewritten per sample, borders never touched)
            vpad_b = [[cpool.tile([P, VPAD_N], BF16, name=f"vpad{i}_{ct}")
                       for ct in range(CT)] for i in range(2)]
            for i in range(2):
                for ct in range(CT):
                    vpv = vpad_b[i][ct][:, 2:2 + PW * PW].rearrange(
                        "p (h w) -> p h w", h=PW)
                    nc.gpsimd.memset(vpv[:, 0:1, :], 0.0)
                    nc.gpsimd.memset(vpv[:, PW - 1:PW, :], 0.0)
                    nc.gpsimd.memset(vpv[:, 1:PW - 1, 0:1], 0.0)
                    nc.gpsimd.memset(vpv[:, 1:PW - 1, PW - 1:PW], 0.0)

            # ---------------- per-sample stage bodies ----------------
            state = {}

            def taps_for(b):
                """Last sample: shift taps {1,7} DVE->PE — at pipeline drain
                the PE is idle while the DVE tap chain gates the final proj."""
                if b == B_LOC - 1:
                    return PE_TAPS + [1, 7], [t for t in DVE_TAPS
                                              if t not in (1, 7)]
                return PE_TAPS, DVE_TAPS

            def stage_a(b):
                """x dma, pool1 (Pool), value conv (PE), evac -> vpad (ACT)."""
                vpads = vpad_b[b % 2]
                pool1 = spool.tile([P, CT, 54, 3], F32, name=f"pool1_{b}",
                                   tag="pool1")
                xchs = []
                for g in range(3):
                    xch = xpool.tile([P, CT, BIG], BF16, name=f"x_{b}_{g}",
                                     tag="xch")
                    xchs.append(xch)
                    nc.sync.dma_start(
                        xch[:],
                        x_d[b, :, :, g * BIG:(g + 1) * BIG].transpose([1, 0, 2]))
                for g in range(3):
                    xch = xchs[g]
                    for kt in range(CT):
                        nc.vector.tensor_reduce(
                            out=pool1[:, kt, g * 18:(g + 1) * 18, :],
                            in_=xch[:, kt].rearrange(
                                "p (h wb w) -> p h wb w", wb=3, w=18),
                            axis=AX.X, op=ALU.add)
                    for mt in range(CT):
                        ps = mmpool.tile([P, 2, 512], F32,
                                         name=f"vps_{b}_{g}_{mt}", tag="mm")
                        for kt in range(CT):
                            for s in range(2):
                                nc.tensor.matmul(
                                    ps[:, s, :CH],
                                    lhsT=wv[:, kt, mt * P:(mt + 1) * P],
                                    rhs=xch[:, kt, s * CH:(s + 1) * CH],
                                    start=(kt == 0), stop=(kt == CT - 1))
                        nc.scalar.activation(
                            out=vpads[mt][:, 2:2 + PW * PW].rearrange(
                                "p (h w) -> p h w", h=PW)[
                                :, 1 + g * 18:1 + (g + 1) * 18, 1:55],
                            in_=ps[:, :, :CH],
                            func=ACTF.Identity, bias=bv[:, mt:mt + 1], scale=1.0)
                state[b] = {"pool1": pool1}

            def stage_b(b):
                """kernel-gen chain + kdiag (ACT)."""
                pool1 = state[b]["pool1"]
                pooled = spool.tile([P, CT, 9], F32, name=f"pooled_{b}",
                                    tag="pooled")
                for kt in range(CT):
                    nc.vector.tensor_reduce(
                        out=pooled[:, kt].rearrange("p (hb wb) -> p hb wb", hb=3),
                        in_=pool1[:, kt].rearrange(
                            "p (hb hs) wb -> p hb wb hs", hb=3),
                        axis=AX.X, op=ALU.add)

                qk = spool.tile([P, CT, 9], F32, name=f"qk_{b}", tag="qk")
                for mt in range(CT):
                    psq = sppool.tile([P, 9], F32, name=f"qps_{b}_{mt}",
                                      tag="sps")
                    for kt in range(CT):
                        nc.tensor.matmul(
                            psq[:],
                            lhsT=wqk[:, kt, mt * P:(mt + 1) * P],
                            rhs=pooled[:, kt],
                            start=(kt == 0), stop=(kt == CT - 1))
                    nc.scalar.activation(out=qk[:, mt], in_=psq[:],
                                         func=ACTF.Identity,
                                         bias=bqk[:, mt:mt + 1], scale=1.0)

                hsb = spool.tile([DQ + 1, 9], F32, name=f"h_{b}", tag="h")
                psh = sppool.tile([DQ, 9], F32, name=f"hps_{b}", tag="sps")
                for kt in range(CT):
                    nc.tensor.matmul(
                        psh[:],
                        lhsT=wkg1[:, kt, :],
                        rhs=qk[:, kt],
                        start=(kt == 0), stop=(kt == CT - 1))
                nc.scalar.activation(out=hsb[:DQ, :], in_=psh[:], func=ACTF.Gelu,
                                     bias=bkg1[:, 0:1], scale=1.0)
                nc.gpsimd.memset(hsb[DQ:DQ + 1, :], 1.0)  # bias row for kg2

                ksb = spool.tile([P, CT, 9], F32, name=f"k_{b}", tag="ksb")
                ksum = spool.tile([P, CT], F32, name=f"ksum_{b}", tag="ksum")
                for mt in range(CT):
                    psk = sppool.tile([P, 9], F32, name=f"kps_{b}_{mt}",
                                      tag="sps")
                    nc.tensor.matmul(
                        psk[:],
                        lhsT=wkg2e[:, mt * P:(mt + 1) * P],
                        rhs=hsb[:],
                        start=True, stop=True)
                    nc.vector.tensor_reduce(out=ksum[:, mt:mt + 1], in_=psk[:],
                                            axis=AX.X, op=ALU.add)
                    nc.vector.tensor_scalar(
                        out=ksum[:, mt:mt + 1], in0=ksum[:, mt:mt + 1],
                        scalar1=fac9[:, mt:mt + 1], scalar2=None, op0=ALU.mult)
                    nc.vector.tensor_scalar(
                        out=ksb[:, mt], in0=psk[:],
                        scalar1=ksum[:, mt:mt + 1], scalar2=None,
                        op0=ALU.subtract)

                # diag(k) for PE taps — on ACT (Copy with per-partition scale)
                pe_taps, _ = taps_for(b)
                kdiag = spool.tile([P, CT, len(PE_TAPS) + 2, P], BF16,
                                   name=f"kd_{b}", tag="kdiag")
                for ct in range(CT):
                    for ti, t in enumerate(pe_taps):
                        nc.scalar.activation(
                            out=kdiag[:, ct, ti], in_=eye[:],
                            func=ACTF.Copy, bias=0.0,
                            scale=ksb[:, ct, t:t + 1])

                state[b].update({"ksb": ksb, "kdiag": kdiag})

            def stage_c(b):
                """depthwise: PE diag taps -> psum -> ACT -> dw; DVE taps as
                tensor_scalar scale into tmp + tensor_tensor add into dw."""
                vpads = vpad_b[b % 2]
                ksb = state[b]["ksb"]
                kdiag = state[b]["kdiag"]
                dw = dwpool.tile([P, CT, PW * PW], BF16, name=f"dw_{b}",
                                 tag="dw")
                pe_taps, dve_taps = taps_for(b)
                tmps = {}
                for ct in range(CT):
                    vpv = vpads[ct][:, 2:2 + PW * PW].rearrange(
                        "p (h w) -> p h w", h=PW)
                    for g in range(3):
                        dps = mmpool.tile([P, 2, 512], F32,
                                          name=f"dps_{b}_{ct}_{g}", tag="mm")
                        for ti, t in enumerate(pe_taps):
                            i, j = divmod(t, 3)
                            for s2 in range(2):
                                ch = g * 2 + s2
                                nc.tensor.matmul(
                                    dps[:, s2, :CH],
                                    lhsT=kdiag[:, ct, ti],
                                    rhs=vpv[:, ch * 9 + i: ch * 9 + i + 9,
                                            j:j + 54],
                                    start=(ti == 0),
                                    stop=(ti == len(pe_taps) - 1))
                        nc.scalar.activation(
                            out=dw[:, ct].rearrange("p (h w) -> p h w", h=PW)[
                                :, 1 + g * 18:1 + (g + 1) * 18, 1:55],
                            in_=dps[:, :, :CH],
                            func=ACTF.Copy, bias=0.0, scale=1.0)
                    # scales first (only need vpad; overlap with ACT evacs)
                    for t in dve_taps:
                        i, j = divmod(t, 3)
                        off = (i - 1) * PW + (j - 1)
                        a0 = max(0, -off)
                        ln = PW * PW - a0
                        tmp = tpool.tile([P, PW * PW], BF16,
                                         name=f"tmp_{b}_{ct}_{t}", tag="tmp")
                        nc.vector.tensor_scalar(
                            out=tmp[:, :ln],
                            in0=vpads[ct][:, 2 + a0 + off: 2 + a0 + off + ln],
                            scalar1=ksb[:, ct, t:t + 1], scalar2=None,
                            op0=ALU.mult)
                        tmps[(ct, t)] = (tmp, a0, ln)
                # adds after evacs: dw[ct] += tmp. Taps in DMA_ADD_TAPS go
                # through the gpsimd software-DGE DMA accumulate path (frees
                # DVE cycles; DMA engines are otherwise ~35% busy).
                for ct in range(CT):
                    for t in dve_taps:
                        tmp, a0, ln = tmps[(ct, t)]
                        if t in DMA_ADD_TAPS:
                            nc.gpsimd.dma_start(
                                dw[:, ct, a0:a0 + ln], tmp[:, :ln],
                                accum_op=ALU.add)
                        else:
                            nc.vector.tensor_tensor(
                                out=dw[:, ct, a0:a0 + ln],
                                in0=dw[:, ct, a0:a0 + ln],
                                in1=tmp[:, :ln], op=ALU.add)
                state[b]["dw"] = dw

            def stage_d(b):
                """proj conv (PE) -> ACT (bias, bf16) -> dma out."""
                dw = state[b]["dw"]
                for g in range(3):
                    for mt in range(CT):
                        ps = mmpool.tile([P, 2, 512], F32,
                                         name=f"pps_{b}_{mt}_{g}", tag="mm")
                        for kt in range(CT):
                            for s in range(2):
                                ch = g * 2 + s
                                nc.tensor.matmul(
                                    ps[:, s, :CH],
                                    lhsT=wproj[:, kt, mt * P:(mt + 1) * P],
                                    rhs=dw[:, kt].rearrange(
                                        "p (h w) -> p h w", h=PW)[
                                        :, ch * 9 + 1: ch * 9 + 10, 1:55],
                                    start=(kt == 0), stop=(kt == CT - 1))
                        ych = ypool.tile([P, BIG], BF16, name=f"y_{b}_{mt}_{g}",
                                         tag="ych")
                        nc.scalar.activation(out=ych[:], in_=ps[:, :, :CH],
                                             func=ACTF.Identity,
                                             bias=bproj[:, mt:mt + 1], scale=1.0)
                        nc.sync.dma_start(
                            y_d[b, mt, :, g * BIG:(g + 1) * BIG], ych[:])

            # ---------------- software-pipelined emission ----------------
            # B(b) sits right before C(b) so its small serial qk chain never
            # head-of-line blocks a proj stage in the in-order PE queue.
            stage_a(0)
            for t_sb, t_dr in late_consts:
                nc.sync.dma_start(t_sb[:], t_dr[:])
            stage_b(0)
            stage_a(1)
            stage_c(0)
            stage_a(2)
            stage_d(0)
            stage_b(1)
            stage_c(1)
            stage_a(3)
            stage_d(1)
            stage_b(2)
            stage_c(2)
            stage_d(2)
            stage_b(3)
            stage_c(3)
            stage_d(3)
    nc.compile()
    return nc


def _prep_inputs(x, w_qk, b_qk, w_kg1, b_kg1, w_kg2, b_kg2, w_v, b_v,
                 w_proj, b_proj, beta):
    bf = ml_dtypes.bfloat16
    f32 = np.float32

    def lay_w(w, dt):  # (O, Cin) -> lhsT layout [p, kt, O]
        wt = np.ascontiguousarray(w.T.reshape(CT, P, -1).transpose(1, 0, 2))
        return wt.astype(dt)

    def lay_b(v):  # (C,) -> [p, ct]
        return np.ascontiguousarray(v.reshape(CT, P).T).astype(f32)

    consts = {
        "wv": lay_w(w_v, bf),
        "wproj": lay_w(w_proj, bf),
        "wqk": lay_w(w_qk / 324.0, f32),
        "wkg1": lay_w(w_kg1, f32),
        "wkg2e": np.ascontiguousarray(
            np.vstack([w_kg2.T, b_kg2[None, :]])).astype(f32),
        "bv": lay_b(b_v),
        "bqk": lay_b(b_qk),
        "bkg1": np.ascontiguousarray(b_kg1.reshape(DQ, 1)).astype(f32),
        "bproj": lay_b(b_proj),
        "fac9": lay_b(1.0 / (1.0 + np.exp(-beta.astype(np.float64))) / 9.0),
        "eye": np.eye(P, dtype=bf),
    }
    xs = np.ascontiguousarray(
        x.reshape(8, B_LOC, CT, P, HW)).astype(bf)
    in_maps = [dict(consts, x=np.ascontiguousarray(xs[c])) for c in range(8)]
    return in_maps


_CACHED_NC = None


def kernel(**inputs):
    global _CACHED_NC
    in_maps = _prep_inputs(**{k: np.asarray(v) for k, v in inputs.items()})
    if _CACHED_NC is None:
        _CACHED_NC = build_program()
    res = run_bass_kernel_spmd(_CACHED_NC, in_maps, core_ids=list(range(8)))
    ys = np.stack([r["y"] for r in res.results])  # (8, 4, 3, 128, 2916)
    return ys.reshape(32, 384, 54, 54).astype(np.float32)
